# revision 9
# baseline (speedup 1.0000x reference)
"""Trainium2 Bass kernel for nn_BracketMultitaskNet.

Computes, for src [S, B, D] (S=256, B=2048, D=128):
    h    = gelu(src @ W1 + b1)                  # [S,B,2D]
    tf   = h @ W2 + b2                          # [S,B,D]
    r    = gelu(tf) + src
    ret  = LayerNorm(r) * gamma + beta          # gamma/beta applied host-side
    diffs[i] = gelu(cat(src[i], src[i+1]) @ Wb + bb)   # [S-1,B,D]
    conditions_results = zeros(3)

Sharding: data-parallel along B across 8 NeuronCores (B_c = 256 per core).
Device layout: activations are kept feature-major ([D on partitions, tokens
on free]) through the matmuls; src is pre-transposed on the host to
[S, D, B_c] so no on-device input transpose is needed.  diffs is produced
feature-major and un-transposed on the host.  ret needs LayerNorm with
per-token scalars, so r is PE-transposed to token-major on device and ret is
stored token-major directly.

Matmul inputs use the f32r (TF32-like) PE fast path: ~1.5e-4 matmul rel err,
4x faster than full fp32.  LayerNorm rsqrt runs on the Vector engine via the
int32 bit-trick + 3 Newton iterations (the ACT engine's sqrt lives in a
different activation-table set than gelu; mixing would thrash table loads).
"""

from contextlib import ExitStack

import numpy as np

S, B, D = 256, 2048, 128
N_CORES = 8
BC = B // N_CORES          # tokens per step per core = 256
SB = 2                     # steps per block
TB = SB * BC               # tokens per block = 512
NB = S // SB               # blocks = 128
NT = TB // 128             # 128-token tiles per block = 4
EPS = 1e-5

_CACHE = {}


def _build(n_steps):
    import concourse.bass as bass
    import concourse.tile as tile
    from concourse import bacc, masks, mybir

    F32 = mybir.dt.float32
    F32R = mybir.dt.float32r
    I32 = mybir.dt.int32
    OP = mybir.AluOpType
    GELU = mybir.ActivationFunctionType.Gelu

    nb = n_steps // SB
    nc = bacc.Bacc("TRN2", target_bir_lowering=False, debug=False,
                   enable_asserts=False, num_devices=N_CORES)

    srcT = nc.dram_tensor("srcT", [n_steps, D, BC], F32R, kind="ExternalInput").ap()
    W1 = nc.dram_tensor("W1", [D, 2 * D], F32R, kind="ExternalInput").ap()
    b1 = nc.dram_tensor("b1", [2 * D], F32, kind="ExternalInput").ap()
    W2 = nc.dram_tensor("W2", [2 * D, D], F32R, kind="ExternalInput").ap()
    b2 = nc.dram_tensor("b2", [D], F32, kind="ExternalInput").ap()
    Wb = nc.dram_tensor("Wb", [2 * D, D], F32R, kind="ExternalInput").ap()
    bb = nc.dram_tensor("bb", [D], F32, kind="ExternalInput").ap()
    ret_d = nc.dram_tensor("ret", [n_steps, BC, D], F32, kind="ExternalOutput").ap()
    diffs_d = nc.dram_tensor("diffsT", [n_steps - 1, D, BC], F32,
                             kind="ExternalOutput").ap()

    with tile.TileContext(nc) as tc, ExitStack() as ctx:
        wpool = ctx.enter_context(tc.tile_pool(name="wpool", bufs=1))
        xpool = ctx.enter_context(tc.tile_pool(name="xpool", bufs=3))
        hpool = ctx.enter_context(tc.tile_pool(name="hpool", bufs=2))
        opool = ctx.enter_context(tc.tile_pool(name="opool", bufs=2))
        spool = ctx.enter_context(tc.tile_pool(name="spool", bufs=2))
        ph = ctx.enter_context(tc.tile_pool(name="ph", bufs=1, space="PSUM"))
        pt = ctx.enter_context(tc.tile_pool(name="pt", bufs=2, space="PSUM"))
        pr = ctx.enter_context(tc.tile_pool(name="pr", bufs=2, space="PSUM"))
        pd = ctx.enter_context(tc.tile_pool(name="pd", bufs=1, space="PSUM"))
        pm = ctx.enter_context(tc.tile_pool(name="pm", bufs=1, space="PSUM"))

        # --- constants / weights (loaded once) ---
        w1sb = wpool.tile([128, 2 * D], F32R)
        nc.sync.dma_start(w1sb[:], W1[:])
        w2sb = wpool.tile([128, 2 * D], F32R)
        nc.sync.dma_start(w2sb[:, 0:D], W2[0:128])
        nc.sync.dma_start(w2sb[:, D:2 * D], W2[128:256])
        wbsb = wpool.tile([128, 2 * D], F32R)
        nc.sync.dma_start(wbsb[:, 0:D], Wb[0:128])
        nc.sync.dma_start(wbsb[:, D:2 * D], Wb[128:256])
        b1sb = wpool.tile([128, 2], F32)
        nc.sync.dma_start(b1sb[:], b1.rearrange("(j k) -> k j", k=128))
        b2sb = wpool.tile([128, 1], F32)
        nc.sync.dma_start(b2sb[:], b2.rearrange("(k o) -> k o", o=1))
        bbsb = wpool.tile([128, 1], F32)
        nc.sync.dma_start(bbsb[:], bb.rearrange("(k o) -> k o", o=1))
        ident = wpool.tile([128, 128], F32)
        masks.make_identity(nc, ident[:])
        ones = wpool.tile([128, 1], F32)
        nc.vector.memset(ones[:], 1.0)

        prev_xT = None
        for b in range(nb):
            s0 = SB * b
            # ---- load src block, feature-major [d, (step tok)] ----
            xT = xpool.tile([128, TB], F32R)
            nc.sync.dma_start(
                xT[:].rearrange("d (s t) -> d s t", s=SB),
                srcT[s0:s0 + SB].rearrange("s d t -> d s t"))
            xf = xT[:].bitcast(F32)  # full-precision bits for the residual

            # ---- mm1 + gelu -> hg [128, (j steptok)] in f32r ----
            h_ps = ph.tile([128, 2 * TB], F32)
            for j in range(2):
                nc.tensor.matmul(h_ps[:, TB * j:TB * (j + 1)],
                                 w1sb[:, 128 * j:128 * (j + 1)], xT[:],
                                 start=True, stop=True)
            hg = hpool.tile([128, 2 * TB], F32R)
            for j in range(2):
                nc.scalar.activation(hg[:, TB * j:TB * (j + 1)],
                                     h_ps[:, TB * j:TB * (j + 1)],
                                     GELU, bias=b1sb[:, j:j + 1])

            # ---- mm2 (accumulate over hidden chunks) + gelu ----
            tf_ps = pt.tile([128, TB], F32)
            for j in range(2):
                nc.tensor.matmul(tf_ps[:], w2sb[:, 128 * j:128 * (j + 1)],
                                 hg[:, TB * j:TB * (j + 1)],
                                 start=(j == 0), stop=(j == 1))
            g = opool.tile([128, TB], F32)
            nc.scalar.activation(g[:], tf_ps[:], GELU, bias=b2sb[:, 0:1])

            # ---- residual (feature-major) on the Pool engine ----
            rT = opool.tile([128, TB], F32)
            nc.gpsimd.tensor_tensor(rT[:], g[:], xf, OP.add)

            # ---- r^2 (Pool engine, feature-major, SBUF) ----
            sqT = opool.tile([128, TB], F32)
            nc.gpsimd.tensor_tensor(sqT[:], rT[:], rT[:], OP.mult)

            # ---- per-token sum and sumsq via PE (chunk.T @ ones) ----
            mu_ps = pm.tile([128, 2 * NT], F32)
            for k in range(NT):
                nc.tensor.matmul(mu_ps[:, k:k + 1],
                                 rT[:, 128 * k:128 * (k + 1)], ones[:],
                                 start=True, stop=True)
                nc.tensor.matmul(mu_ps[:, NT + k:NT + k + 1],
                                 sqT[:, 128 * k:128 * (k + 1)], ones[:],
                                 start=True, stop=True)

            # ---- transpose r to token-major ----
            rtok = pr.tile([128, TB], F32)
            for k in range(NT):
                nc.tensor.transpose(rtok[:, 128 * k:128 * (k + 1)],
                                    rT[:, 128 * k:128 * (k + 1)], ident[:])

            # ---- LayerNorm scalars: mu/128, rsqrt(var+eps) ----
            mu = spool.tile([128, 2 * NT], F32)
            nc.vector.tensor_copy(mu[:], mu_ps[:])
            ssq = mu[:, NT:2 * NT]
            musc = spool.tile([128, NT], F32)
            nc.vector.tensor_scalar(musc[:], mu[:, 0:NT], 1.0 / D, None, OP.mult)
            msq = spool.tile([128, NT], F32)
            nc.vector.tensor_tensor(msq[:], musc[:], musc[:], OP.mult)
            nc.vector.tensor_scalar(msq[:], msq[:], EPS, None, OP.subtract)
            v = spool.tile([128, NT], F32)
            nc.vector.scalar_tensor_tensor(v[:], ssq, 1.0 / D, msq[:],
                                           OP.mult, OP.subtract)
            # rsqrt(v): int32 bit-trick init + 3 Newton iterations
            y = spool.tile([128, NT], F32)
            ti = spool.tile([128, NT], I32)
            nc.vector.tensor_scalar(ti[:], v[:].bitcast(I32), 1, None,
                                    OP.logical_shift_right)
            nc.vector.tensor_scalar(y[:].bitcast(I32), ti[:], 0x5F3759DF, -1,
                                    OP.subtract, OP.mult)
            hw_ = spool.tile([128, NT], F32)
            t2 = spool.tile([128, NT], F32)
            for _ in range(3):
                nc.vector.tensor_tensor(hw_[:], v[:], y[:], OP.mult)
                nc.vector.scalar_tensor_tensor(t2[:], hw_[:], -0.5, y[:],
                                               OP.mult, OP.mult)
                nc.vector.scalar_tensor_tensor(y[:], t2[:], 1.5, y[:],
                                               OP.add, OP.mult)

            # ---- apply LN + store ret (token-major) ----
            ret_sb = opool.tile([128, NT, 128], F32)
            for k in range(NT):
                nc.vector.tensor_scalar(ret_sb[:, k], rtok[:, 128 * k:128 * (k + 1)],
                                        musc[:, k:k + 1], y[:, k:k + 1],
                                        OP.subtract, OP.mult)
            nc.sync.dma_start(
                ret_d[s0:s0 + SB].rearrange("s (k p) d -> p (s k) d", p=128),
                ret_sb[:])

            # ---- diffs: pairs (s0-1, s0) and (s0, s0+1) ----
            d_ps = pd.tile([128, TB], F32)
            if b > 0:
                nc.tensor.matmul(d_ps[:, 0:BC], wbsb[:, 0:D],
                                 prev_xT[:, BC:TB], start=True, stop=False)
                nc.tensor.matmul(d_ps[:, 0:BC], wbsb[:, D:2 * D],
                                 xT[:, 0:BC], start=False, stop=True)
            nc.tensor.matmul(d_ps[:, BC:TB], wbsb[:, 0:D],
                             xT[:, 0:BC], start=True, stop=False)
            nc.tensor.matmul(d_ps[:, BC:TB], wbsb[:, D:2 * D],
                             xT[:, BC:TB], start=False, stop=True)
            dg = opool.tile([128, TB], F32)
            if b > 0:
                nc.scalar.activation(dg[:], d_ps[:], GELU, bias=bbsb[:, 0:1])
                nc.sync.dma_start(
                    diffs_d[s0 - 1:s0 + 1].rearrange("q d t -> d q t"),
                    dg[:].rearrange("d (q t) -> d q t", q=2))
            else:
                nc.scalar.activation(dg[:, BC:TB], d_ps[:, BC:TB], GELU,
                                     bias=bbsb[:, 0:1])
                nc.sync.dma_start(
                    diffs_d[0:1].rearrange("q d t -> d q t"),
                    dg[:, BC:TB].rearrange("d (q t) -> d q t", q=1))

            prev_xT = xT

    nc.compile()
    return nc


def _get_nc(n_steps):
    if n_steps not in _CACHE:
        _CACHE[n_steps] = _build(n_steps)
    return _CACHE[n_steps]


def kernel(src, Wb, bb, W1, b1, W2, b2, gamma, beta, _trace=False):
    from concourse.bass_utils import run_bass_kernel_spmd

    src = np.asarray(src, dtype=np.float32)
    n_steps = src.shape[0]
    nc = _get_nc(n_steps)

    weights = {
        "W1": np.asarray(W1, np.float32), "b1": np.asarray(b1, np.float32),
        "W2": np.asarray(W2, np.float32), "b2": np.asarray(b2, np.float32),
        "Wb": np.asarray(Wb, np.float32), "bb": np.asarray(bb, np.float32),
    }
    in_maps = []
    for c in range(N_CORES):
        shard = src[:, c * BC:(c + 1) * BC, :]          # [S, BC, D]
        srcT = np.ascontiguousarray(shard.transpose(0, 2, 1))  # [S, D, BC]
        in_maps.append({"srcT": srcT, **weights})

    res = run_bass_kernel_spmd(nc, in_maps, core_ids=list(range(N_CORES)),
                               trace=_trace)

    ret = np.empty((n_steps, src.shape[1], D), np.float32)
    diffs = np.empty((n_steps - 1, src.shape[1], D), np.float32)
    for c, r in enumerate(res.results):
        sl = slice(c * BC, (c + 1) * BC)
        ret[:, sl, :] = r["ret"]
        diffs[:, sl, :] = r["diffsT"].transpose(0, 2, 1)

    gamma = np.asarray(gamma, np.float32)
    beta = np.asarray(beta, np.float32)
    if not (np.all(gamma == 1.0) and np.all(beta == 0.0)):
        ret = ret * gamma + beta

    conditions_results = np.zeros(3, dtype=src.dtype)
    out = (ret, diffs, conditions_results)
    if _trace:
        return out, res
    return out


# revision 17
# speedup vs baseline: 1.5939x; 1.5939x over previous
"""Trainium2 Bass kernel for nn_BracketMultitaskNet.

Computes, for src [S, B, D] (S=256, B=2048, D=128):
    h    = gelu(src @ W1 + b1)                  # [S,B,2D]
    tf   = h @ W2 + b2                          # [S,B,D]
    r    = gelu(tf) + src
    ret  = LayerNorm(r) * gamma + beta          # gamma/beta applied host-side
    diffs[i] = gelu(cat(src[i], src[i+1]) @ Wb + bb)   # [S-1,B,D]
    conditions_results = zeros(3)

Sharding: data-parallel along B across 8 NeuronCores (B_c = 256 per core).
Device layout: activations are kept feature-major ([D on partitions, tokens
on free]) through the matmuls; src is pre-transposed on the host to
[S, D, B_c] so no on-device input transpose is needed.  diffs is produced
feature-major and un-transposed on the host.  ret needs LayerNorm with
per-token scalars, so r is PE-transposed to token-major on device and ret is
stored token-major directly.

Matmul inputs use the f32r (TF32-like) PE fast path: ~1.5e-4 matmul rel err,
4x faster than full fp32.  LayerNorm rsqrt runs on the Vector engine via the
int32 bit-trick + 3 Newton iterations (the ACT engine's sqrt lives in a
different activation-table set than gelu; mixing would thrash table loads).
"""

from contextlib import ExitStack

import numpy as np

S, B, D = 256, 2048, 128
N_CORES = 8
BC = B // N_CORES          # tokens per step per core = 256
SB = 2                     # steps per block
TB = SB * BC               # tokens per block = 512
NB = S // SB               # blocks = 128
NT = TB // 128             # 128-token tiles per block = 4
EPS = 1e-5

_CACHE = {}


def _build(n_steps):
    import concourse.bass as bass
    import concourse.tile as tile
    from concourse import bacc, masks, mybir

    F32 = mybir.dt.float32
    F32R = mybir.dt.float32r
    I32 = mybir.dt.int32
    OP = mybir.AluOpType
    GELU = mybir.ActivationFunctionType.Gelu

    nb = n_steps // SB
    nc = bacc.Bacc("TRN2", target_bir_lowering=False, debug=False,
                   enable_asserts=False, num_devices=N_CORES)

    srcT = nc.dram_tensor("srcT", [n_steps, D, BC], F32R, kind="ExternalInput").ap()
    W1 = nc.dram_tensor("W1", [D, 2 * D], F32R, kind="ExternalInput").ap()
    b1 = nc.dram_tensor("b1", [2 * D], F32, kind="ExternalInput").ap()
    W2 = nc.dram_tensor("W2", [2 * D, D], F32R, kind="ExternalInput").ap()
    b2 = nc.dram_tensor("b2", [D], F32, kind="ExternalInput").ap()
    Wb = nc.dram_tensor("Wb", [2 * D, D], F32R, kind="ExternalInput").ap()
    bb = nc.dram_tensor("bb", [D], F32, kind="ExternalInput").ap()
    ret_d = nc.dram_tensor("ret", [n_steps, BC, D], F32, kind="ExternalOutput").ap()
    diffs_d = nc.dram_tensor("diffsT", [n_steps - 1, D, BC], F32,
                             kind="ExternalOutput").ap()

    with tile.TileContext(nc) as tc, ExitStack() as ctx:
        wpool = ctx.enter_context(tc.tile_pool(name="wpool", bufs=1))
        xpool = ctx.enter_context(tc.tile_pool(name="xpool", bufs=3))
        hpool = ctx.enter_context(tc.tile_pool(name="hpool", bufs=2))
        opool = ctx.enter_context(tc.tile_pool(name="opool", bufs=2))
        rpool = ctx.enter_context(tc.tile_pool(name="rpool", bufs=6))
        spool = ctx.enter_context(tc.tile_pool(name="spool", bufs=2))
        ph = ctx.enter_context(tc.tile_pool(name="ph", bufs=1, space="PSUM"))
        pt = ctx.enter_context(tc.tile_pool(name="pt", bufs=2, space="PSUM"))
        pr = ctx.enter_context(tc.tile_pool(name="pr", bufs=1, space="PSUM"))
        pd = ctx.enter_context(tc.tile_pool(name="pd", bufs=2, space="PSUM"))

        # --- constants / weights (loaded once) ---
        w1sb = wpool.tile([128, 2 * D], F32R)
        nc.sync.dma_start(w1sb[:], W1[:])
        w2sb = wpool.tile([128, 2 * D], F32R)
        nc.sync.dma_start(w2sb[:, 0:D], W2[0:128])
        nc.sync.dma_start(w2sb[:, D:2 * D], W2[128:256])
        wbsb = wpool.tile([128, 2 * D], F32R)
        nc.sync.dma_start(wbsb[:, 0:D], Wb[0:128])
        nc.sync.dma_start(wbsb[:, D:2 * D], Wb[128:256])
        b1sb = wpool.tile([128, 2], F32)
        nc.sync.dma_start(b1sb[:], b1.rearrange("(j k) -> k j", k=128))
        b2sb = wpool.tile([128, 1], F32)
        nc.sync.dma_start(b2sb[:], b2.rearrange("(k o) -> k o", o=1))
        bbsb = wpool.tile([128, 1], F32)
        nc.sync.dma_start(bbsb[:], bb.rearrange("(k o) -> k o", o=1))
        ident = wpool.tile([128, 128], F32)
        masks.make_identity(nc, ident[:])

        prev_xT = None
        SBB = 4                       # blocks per stats super-block
        pend = []                     # (r_sb, s0, col) awaiting apply
        musb = ssqb = None
        for b in range(nb):
            s0 = SB * b
            # ---- load src block, feature-major [d, (step tok)] ----
            xT = xpool.tile([128, TB], F32R)
            nc.sync.dma_start(
                xT[:].rearrange("d (s t) -> d s t", s=SB),
                srcT[s0:s0 + SB].rearrange("s d t -> d s t"))
            xf = xT[:].bitcast(F32)  # full-precision bits for the residual

            # ---- mm1 + gelu -> hg [128, (j steptok)] in f32r ----
            h_ps = ph.tile([128, 2 * TB], F32)
            for j in range(2):
                nc.tensor.matmul(h_ps[:, TB * j:TB * (j + 1)],
                                 w1sb[:, 128 * j:128 * (j + 1)], xT[:],
                                 start=True, stop=True)
            hg = hpool.tile([128, 2 * TB], F32R)
            for j in range(2):
                nc.scalar.activation(hg[:, TB * j:TB * (j + 1)],
                                     h_ps[:, TB * j:TB * (j + 1)],
                                     GELU, bias=b1sb[:, j:j + 1])

            # ---- mm2 (accumulate over hidden chunks) + gelu ----
            tf_ps = pt.tile([128, TB], F32)
            for j in range(2):
                nc.tensor.matmul(tf_ps[:], w2sb[:, 128 * j:128 * (j + 1)],
                                 hg[:, TB * j:TB * (j + 1)],
                                 start=(j == 0), stop=(j == 1))
            g = opool.tile([128, TB], F32)
            nc.scalar.activation(g[:], tf_ps[:], GELU, bias=b2sb[:, 0:1])

            # ---- residual (feature-major) on the Pool engine ----
            rT = opool.tile([128, TB], F32)
            nc.gpsimd.tensor_tensor(rT[:], g[:], xf, OP.add)

            # ---- transpose r to token-major ----
            rtA = pr.tile([128, 2, 128], F32, tag="rtA")
            rtB = pr.tile([128, 2, 128], F32, tag="rtB")
            for k in range(NT):
                dst = (rtA if k < 2 else rtB)[:, k % 2]
                nc.tensor.transpose(dst, rT[:, 128 * k:128 * (k + 1)], ident[:])

            # ---- move r to SBUF ----
            r_sb = rpool.tile([128, 4, 128], F32)
            nc.vector.tensor_copy(r_sb[:, 0:2], rtA[:])
            nc.vector.tensor_copy(r_sb[:, 2:4], rtB[:])

            # ---- stats accumulation arrays (per super-block of SBB blocks) --
            bb_i = b % SBB
            if bb_i == 0:
                musb = spool.tile([128, SBB * NT], F32, tag="musb")
                ssqb = spool.tile([128, SBB * NT], F32, tag="ssqb")
            # per-token mean-sums: one 3D group-reduce over the feature axis
            nc.vector.tensor_reduce(musb[:, NT * bb_i:NT * (bb_i + 1)],
                                    r_sb[:], mybir.AxisListType.X, OP.add)
            # per-token sum of squares: STT with accumulator, split DVE/Pool
            sqscr = spool.tile([128, 128], F32, tag="sqscr")
            for k in range(NT):
                nc.vector.scalar_tensor_tensor(
                    sqscr[:], r_sb[:, k], 0.0, r_sb[:, k],
                    OP.bypass, OP.mult,
                    accum_out=ssqb[:, NT * bb_i + k:NT * bb_i + k + 1])
            pend.append((r_sb, s0))

            # ---- end of super-block: LN scalars + apply + store ----
            if bb_i == SBB - 1 or b == nb - 1:
                W = NT * len(pend)
                musc = spool.tile([128, SBB * NT], F32, tag="musc")
                nc.vector.tensor_scalar(musc[:, 0:W], musb[:, 0:W],
                                        1.0 / D, None, OP.mult)
                msq = spool.tile([128, SBB * NT], F32, tag="msq")
                nc.vector.tensor_tensor(msq[:, 0:W], musc[:, 0:W], musc[:, 0:W],
                                        OP.mult)
                nc.vector.tensor_scalar(msq[:, 0:W], msq[:, 0:W], EPS, None,
                                        OP.subtract)
                v = spool.tile([128, SBB * NT], F32, tag="v")
                nc.vector.scalar_tensor_tensor(v[:, 0:W], ssqb[:, 0:W], 1.0 / D,
                                               msq[:, 0:W], OP.mult, OP.subtract)
                # rsqrt(v): int32 bit-trick init + 3 Newton iterations
                y = spool.tile([128, SBB * NT], F32, tag="y")
                ti = spool.tile([128, SBB * NT], I32, tag="ti")
                nc.vector.tensor_scalar(ti[:, 0:W], v[:, 0:W].bitcast(I32), 1,
                                        None, OP.logical_shift_right)
                nc.vector.tensor_scalar(y[:, 0:W].bitcast(I32), ti[:, 0:W],
                                        0x5F3759DF, -1, OP.subtract, OP.mult)
                hw_ = spool.tile([128, SBB * NT], F32, tag="hw")
                t2 = spool.tile([128, SBB * NT], F32, tag="t2")
                for _ in range(3):
                    nc.vector.tensor_tensor(hw_[:, 0:W], v[:, 0:W], y[:, 0:W],
                                            OP.mult)
                    nc.vector.scalar_tensor_tensor(t2[:, 0:W], hw_[:, 0:W], -0.5,
                                                   y[:, 0:W], OP.mult, OP.mult)
                    nc.vector.scalar_tensor_tensor(y[:, 0:W], t2[:, 0:W], 1.5,
                                                   y[:, 0:W], OP.add, OP.mult)
                for i, (rsb_i, s0_i) in enumerate(pend):
                    ret_sb = opool.tile([128, NT, 128], F32, tag="ret_sb",
                                        bufs=5)
                    for k in range(NT):
                        c = NT * i + k
                        nc.vector.tensor_scalar(ret_sb[:, k], rsb_i[:, k],
                                                musc[:, c:c + 1], y[:, c:c + 1],
                                                OP.subtract, OP.mult)
                    nc.sync.dma_start(
                        ret_d[s0_i:s0_i + SB].rearrange(
                            "s (k p) d -> p (s k) d", p=128),
                        ret_sb[:])
                pend = []

            # ---- diffs: pairs (s0-1, s0) and (s0, s0+1) ----
            d_ps = pd.tile([128, TB], F32)
            if b > 0:
                nc.tensor.matmul(d_ps[:, 0:BC], wbsb[:, 0:D],
                                 prev_xT[:, BC:TB], start=True, stop=False)
                nc.tensor.matmul(d_ps[:, 0:BC], wbsb[:, D:2 * D],
                                 xT[:, 0:BC], start=False, stop=True)
            nc.tensor.matmul(d_ps[:, BC:TB], wbsb[:, 0:D],
                             xT[:, 0:BC], start=True, stop=False)
            nc.tensor.matmul(d_ps[:, BC:TB], wbsb[:, D:2 * D],
                             xT[:, BC:TB], start=False, stop=True)
            dg = opool.tile([128, TB], F32)
            if b > 0:
                nc.scalar.activation(dg[:], d_ps[:], GELU, bias=bbsb[:, 0:1])
                nc.sync.dma_start(
                    diffs_d[s0 - 1:s0 + 1].rearrange("q d t -> d q t"),
                    dg[:].rearrange("d (q t) -> d q t", q=2))
            else:
                nc.scalar.activation(dg[:, BC:TB], d_ps[:, BC:TB], GELU,
                                     bias=bbsb[:, 0:1])
                nc.sync.dma_start(
                    diffs_d[0:1].rearrange("q d t -> d q t"),
                    dg[:, BC:TB].rearrange("d (q t) -> d q t", q=1))

            prev_xT = xT

    nc.compile()
    return nc


def _get_nc(n_steps):
    if n_steps not in _CACHE:
        _CACHE[n_steps] = _build(n_steps)
    return _CACHE[n_steps]


def kernel(src, Wb, bb, W1, b1, W2, b2, gamma, beta, _trace=False):
    from concourse.bass_utils import run_bass_kernel_spmd

    src = np.asarray(src, dtype=np.float32)
    n_steps = src.shape[0]
    nc = _get_nc(n_steps)

    weights = {
        "W1": np.asarray(W1, np.float32), "b1": np.asarray(b1, np.float32),
        "W2": np.asarray(W2, np.float32), "b2": np.asarray(b2, np.float32),
        "Wb": np.asarray(Wb, np.float32), "bb": np.asarray(bb, np.float32),
    }
    in_maps = []
    for c in range(N_CORES):
        shard = src[:, c * BC:(c + 1) * BC, :]          # [S, BC, D]
        srcT = np.ascontiguousarray(shard.transpose(0, 2, 1))  # [S, D, BC]
        in_maps.append({"srcT": srcT, **weights})

    res = run_bass_kernel_spmd(nc, in_maps, core_ids=list(range(N_CORES)),
                               trace=_trace)

    ret = np.empty((n_steps, src.shape[1], D), np.float32)
    diffs = np.empty((n_steps - 1, src.shape[1], D), np.float32)
    for c, r in enumerate(res.results):
        sl = slice(c * BC, (c + 1) * BC)
        ret[:, sl, :] = r["ret"]
        diffs[:, sl, :] = r["diffsT"].transpose(0, 2, 1)

    gamma = np.asarray(gamma, np.float32)
    beta = np.asarray(beta, np.float32)
    if not (np.all(gamma == 1.0) and np.all(beta == 0.0)):
        ret = ret * gamma + beta

    conditions_results = np.zeros(3, dtype=src.dtype)
    out = (ret, diffs, conditions_results)
    if _trace:
        return out, res
    return out
